# revision 72
# baseline (speedup 1.0000x reference)
"""Expert-parallel MoE layer for Trainium2 (Bass/Tile, 8 NeuronCores).

Strategy (hardcoded for B=4, T=2048, C=1024, E=8, H=2728, top_k=2):
  - Host computes the router (top-2 selection and softmax combine weights)
    and performs the all-to-all token dispatch/combine as the shard/unshard
    step. Selection uses a stable argsort over fp32 logits (verified to
    match jax.lax.top_k for this regime).
  - Balanced two-segment dispatch: each core processes s1 tokens of its
    primary expert (weight set A) plus up to s2 tokens of ONE secondary
    expert (weight set B) -- overloaded experts shed spillover to
    underloaded cores. Matmul time is charged per moving-dim element, so
    splitting chains at the segment boundary costs no PE cycles, and the
    program capacity drops from max-expert-load to near mean-load
    (s1+s2 = 2072 vs 2176 for the seed-0 routing).
  - Each core runs the fused FFN in one pass, all in bf16 (same PE rate as
    float32r, half the DMA/SBUF):
      phase A: s = silu(x@w1.T) * (x@w3.T), RESIDENT in SBUF -- no DRAM
               spill. The two GEMMs' 40-row tails fuse into one 128-row
               chain (43 chains, the packing minimum, instead of 2x22),
               with a small DRAM bounce realigning the p3 half's
               partitions. segB's chains ride the same h-loop and shared
               weight-stream ring, a few levels behind.
      phase B: y = (s @ w2.T) * g, streamed straight to HBM. segA tiles
               use the resident c-major w2; segB runs TRANSPOSED (tokens
               on partitions, C moving) so its w2 streams as cheap
               [128, C] h-tiles, one accumulation step interleaved per
               main chain into two persistent PSUM banks -- fully hidden.
    All PSUM pools are pre-allocated (exactly 8 banks; the PE-warmup
    shares phase B's) because bank reuse would couple phase B's start to
    the tail-pass bounce pipeline.
"""

import os
import sys
from contextlib import ExitStack

import numpy as np

for _p in ("/opt/trn_rl_repo", "/root/.axon_site/_ro/trn_rl_repo"):
    if os.path.isdir(_p) and _p not in sys.path:
        sys.path.insert(0, _p)

import concourse.mybir as mybir
import concourse.tile as tile
from concourse.tile_rust import add_dep_helper
from concourse import bacc
from concourse.bass_utils import run_bass_kernel_spmd

FP32 = mybir.dt.float32
BF16 = mybir.dt.bfloat16
NP_BF16 = mybir.dt.np(mybir.dt.bfloat16)
ALU = mybir.AluOpType
AF = mybir.ActivationFunctionType

E = 8            # experts == cores
C = 1024         # model dim
H0 = 2728        # ffn hidden dim
KC = C // 128    # 8 contraction tiles over C
KH = (H0 + 127) // 128  # 22 tiles over padded H
HP = KH * 128    # 2816
KHF = H0 // 128  # 21 full 128-row h-tiles
TR = H0 - KHF * 128     # 40 remainder h-rows
TW2 = 2 * TR     # fused tail chain: [40 w1-tail rows | 40 w3-tail rows]
TT = 512         # max token tile (fp32 PSUM bank = 512 floats)
W_LOOK = 5       # w1/w3 h-tile buffer depth (slack decouples slot waits)
H_PRE = 4        # leading h-tiles interleaved token-major (paces the x stream)
N_WARM = 36      # PE warmup matmuls covering the p-state ramp at startup
CAP_MAX = 2176   # per-launch token cap (SBUF budget); split into runs beyond

_CACHE = {}
LAST_RESULTS = None


def _token_tiles(cap):
    """Token tiles: a 384 tile first (its x DMA plus the h0 weight pair is
    exactly what the PE's first H_PRE passes cover -- see phase A), full 512
    tiles in the middle, remainder folded into >=256 trailing tiles so every
    x/s/y DMA moves >=512B contiguous runs (below that the DMA pays a 2x
    penalty). bf16 matmuls have no narrow-tile penalty. Phase B iterates
    smallest-last, keeping the final drain short."""
    if cap <= TT:
        widths = [cap]
    else:
        widths = [384]
        r = cap - 384
        while r >= 768:
            widths.append(TT)
            r -= TT
        if r > TT:
            widths += [r - 256, 256]
        else:
            widths.append(r)
    assert sum(widths) == cap and all(256 <= w <= TT for w in widths)
    tiles = []
    off = 0
    for w in widths:
        tiles.append((off, w))
        off += w
    return tiles


def _build(s1, s2):
    """Build + compile the SPMD program: s1 tokens of the core's primary
    expert (weight set A) + s2 tokens of one secondary expert (weight set
    B). Splitting a chain at the segment boundary costs no PE cycles (cost
    is charged per moving-dim element), so balancing experts across cores
    this way shrinks the program capacity from max-expert-load to
    ~mean-load."""
    cap = s1 + s2
    tiles = _token_tiles(s1) + [(s1, s2)]
    nsa = len(tiles) - 1  # segA tile count
    nc = bacc.Bacc("TRN2", target_bir_lowering=False, debug=False, num_devices=E)

    xs = nc.dram_tensor("xs", [KC, 128, cap], BF16, kind="ExternalInput").ap()
    w1s = nc.dram_tensor("w1s", [KHF, 128, C], BF16, kind="ExternalInput").ap()
    w3s = nc.dram_tensor("w3s", [KHF, 128, C], BF16, kind="ExternalInput").ap()
    w1sB = nc.dram_tensor("w1sB", [KHF, 128, C], BF16, kind="ExternalInput").ap()
    w3sB = nc.dram_tensor("w3sB", [KHF, 128, C], BF16, kind="ExternalInput").ap()
    w13sB = nc.dram_tensor("w13sB", [128, KC * 128], BF16,
                           kind="ExternalInput").ap()
    w2bs = nc.dram_tensor("w2bs", [KH, 128, C], BF16, kind="ExternalInput").ap()
    gcs = nc.dram_tensor("gcs", [128, 1], FP32, kind="ExternalInput").ap()
    ytB = nc.dram_tensor("ytB", [128, C], BF16, kind="ExternalOutput").ap()
    w13s = nc.dram_tensor("w13s", [128, KC * 128], BF16,
                          kind="ExternalInput").ap()
    w2s = nc.dram_tensor("w2s", [KC, 128, KH * 128], BF16, kind="ExternalInput").ap()
    gs = nc.dram_tensor("gs", [128, cap], BF16, kind="ExternalInput").ap()
    yt = nc.dram_tensor("yt", [KC, 128, cap], BF16, kind="ExternalOutput").ap()

    with tile.TileContext(nc) as tc, ExitStack() as top:
        # resident tensors
        xp = top.enter_context(tc.tile_pool(name="xres", bufs=1))
        x_sb = xp.tile([128, KC, cap], BF16, name="x_sb")
        w2p = top.enter_context(tc.tile_pool(name="w2res", bufs=1))
        w2_sb = [w2p.tile([128, KH, 128], BF16, tag=f"w2_{c}", name=f"w2_sb{c}")
                 for c in range(KC)]
        sres = top.enter_context(tc.tile_pool(name="sres", bufs=1))
        s_sb = sres.tile([128, KHF, cap], BF16, name="s_sb")
        s_tl = sres.tile([TR, cap], BF16, name="s_tl")
        w13p = top.enter_context(tc.tile_pool(name="w13res", bufs=1))
        w13_sb = w13p.tile([128, KC * 128], BF16, name="w13_sb")
        w13B_sb = w13p.tile([128, KC * 128], BF16, name="w13B_sb")
        gp = top.enter_context(tc.tile_pool(name="gres", bufs=1))
        g_sb = gp.tile([128, cap], BF16, name="g_sb")
        gc_sb = gp.tile([128, 1], FP32, name="gc_sb")

        # Startup: DMA issue costs ~0.65us of SEQ time per dma_start and the
        # DMA device is serial, so the startup-critical stream rides the
        # SYNC queue as few, whole-tile DMAs in exact consumption order:
        # w1[h0], x t0, w3[h0], the h1..h3 pairs, then the remaining x
        # tiles. Later weight pairs are dependency-delayed onto the GPSIMD
        # queue so they cannot cut in front of this stream on the shared
        # DMA device.
        wst = top.enter_context(tc.tile_pool(name="wst", bufs=W_LOOK))
        w_cur = {}

        def load_wh(h, eng=None, dep=None):
            w1_sb = wst.tile([128, C], BF16, tag="w1", name=f"w1_sb{h}")
            w3_sb = wst.tile([128, C], BF16, tag="w3", name=f"w3_sb{h}")
            d1 = (eng or nc.gpsimd).dma_start(w1_sb[:], w1s[h])
            d3 = (eng or nc.gpsimd).dma_start(w3_sb[:], w3s[h])
            if dep is not None:
                add_dep_helper(d1.ins, dep.ins, reason="stagger w stream")
                add_dep_helper(d3.ins, dep.ins, reason="stagger w stream")
            w_cur[h] = (w1_sb, w3_sb)

        xin = xs.rearrange("k p t -> p k t")
        w1_sb0 = wst.tile([128, C], BF16, tag="w1", name="w1_sb0")
        w3_sb0 = wst.tile([128, C], BF16, tag="w3", name="w3_sb0")
        w_cur[0] = (w1_sb0, w3_sb0)
        nc.sync.dma_start(w1_sb0[:], w1s[0])
        to0, tw0 = tiles[0]
        kh = KC // 2
        nc.sync.dma_start(x_sb[:, :kh, to0:to0 + tw0],
                          xin[:, :kh, to0:to0 + tw0])
        nc.sync.dma_start(x_sb[:, kh:, to0:to0 + tw0],
                          xin[:, kh:, to0:to0 + tw0])
        nc.sync.dma_start(w3_sb0[:], w3s[0])
        for h in range(1, min(H_PRE, KH)):
            load_wh(h, eng=nc.sync)
        for to, tw in tiles[1:]:
            nc.sync.dma_start(x_sb[:, :, to:to + tw], xin[:, :, to:to + tw])

        # PE warmup: dependency-free matmuls on a memset tile keep the PE
        # busy through its p-state ramp while the first real DMAs land, so
        # real matmuls start at full clock. Sized to end just as the first
        # weight/x chunks arrive.
        wup = top.enter_context(tc.tile_pool(name="wup", bufs=1))
        wu = wup.tile([128, 128], BF16, name="wu")
        psb = top.enter_context(tc.tile_pool(name="psB", bufs=2, space="PSUM"))
        psbB = top.enter_context(tc.tile_pool(name="psBB", bufs=1, space="PSUM"))
        pyB = [psbB.tile([128, TT], FP32, tag=f"pyB{i}", name=f"pyB{i}")
               for i in range(2)]
        wu_ps = psb.tile([128, TT], FP32, tag="py", name="wu_ps")
        nc.gpsimd.memset(wu[:], 0.0)
        for _ in range(N_WARM):
            nc.tensor.matmul(wu_ps[:, :128], wu[:], wu[:], start=True,
                             stop=True)

        # ---- phase A: s = silu(x@w1.T) * (x@w3.T), resident in SBUF ----
        psa = top.enter_context(tc.tile_pool(name="psA", bufs=2, space="PSUM"))
        sap = top.enter_context(tc.tile_pool(name="sap", bufs=2))
        anchors = {}

        anchors0 = {}

        def emit_ht(h, to, tw, wpair=None):
            w1_sb, w3_sb = wpair or w_cur[h]
            p1 = psa.tile([128, TT], FP32, tag="p1", name=f"p1_{h}")
            for k in range(KC):
                mm = nc.tensor.matmul(p1[:, :tw],
                                      w1_sb[:, k * 128:(k + 1) * 128],
                                      x_sb[:, k, to:to + tw],
                                      start=(k == 0), stop=(k == KC - 1))
                if k == 0:
                    anchors0.setdefault(h, mm)
            p3 = psa.tile([128, TT], FP32, tag="p3", name=f"p3_{h}")
            for k in range(KC):
                mm = nc.tensor.matmul(p3[:, :tw],
                                      w3_sb[:, k * 128:(k + 1) * 128],
                                      x_sb[:, k, to:to + tw],
                                      start=(k == 0), stop=(k == KC - 1))
            anchors[h] = mm
            sa = sap.tile([128, TT], BF16, tag="sa", name=f"sa{h}")
            nc.scalar.activation(sa[:, :tw], p1[:, :tw], AF.Silu)
            nc.vector.tensor_tensor(s_sb[:, h, to:to + tw], sa[:, :tw],
                                    p3[:, :tw], op=ALU.mult)

        w_curB = {}

        def load_whB(h, dep=None):
            w1_sb = wst.tile([128, C], BF16, tag="w1", name=f"w1b_sb{h}")
            w3_sb = wst.tile([128, C], BF16, tag="w3", name=f"w3b_sb{h}")
            d1 = nc.gpsimd.dma_start(w1_sb[:], w1sB[h])
            d3 = nc.gpsimd.dma_start(w3_sb[:], w3sB[h])
            if dep is not None:
                add_dep_helper(d1.ins, dep.ins, reason="stagger wB stream")
                add_dep_helper(d3.ins, dep.ins, reason="stagger wB stream")
            w_curB[h] = (w1_sb, w3_sb)

        # The first H_PRE h-tiles run token-major so the PE's x consumption
        # paces the incoming x stream (x tile t is only needed after H_PRE
        # passes over tiles < t) -- no PE stall while x streams in.
        for ti, (to, tw) in enumerate(tiles[:nsa]):
            for h in range(min(H_PRE, KHF)):
                emit_ht(h, to, tw)
            if ti == 0 and H_PRE < KHF:
                load_wh(H_PRE, dep=anchors[0])
        if H_PRE + 1 < KHF:
            load_wh(H_PRE + 1)
        load_whB(0, dep=anchors[1])
        toB, twB = tiles[nsa]
        wB_loaded = 0

        def segb_plan(h):
            # segB chains double up for the first iterations, then run in
            # lockstep with h -- keeps <=4 live tiles in the shared ring
            if h < 2 * H_PRE:
                return [2 * (h - H_PRE), 2 * (h - H_PRE) + 1]
            return [h] if h < KHF else []

        for h in range(H_PRE, KHF):
            want = segb_plan(h)
            while want and wB_loaded + 1 < KHF and wB_loaded <= want[-1]:
                load_whB(wB_loaded + 1)
                wB_loaded += 1
            for to, tw in tiles[:nsa]:
                emit_ht(h, to, tw)
            for hB in want:
                emit_ht(hB, toB, twB, wpair=w_curB.pop(hB))
            if h + 2 < KHF:
                load_wh(h + 2)
            # w2 / w13 / g loads ride the SCALAR queue (separate from the
            # w1/w3 stream), dependency-anchored to the current h-tile's
            # last matmul so the tile scheduler cannot hoist them into the
            # startup x window; spread every other h iteration so they
            # never head-block the queue or crowd the DMA device.
            if h % 2 == 0 and H_PRE <= h <= H_PRE + 2 * (KC - 1):
                c = (h - H_PRE) // 2
                w2dma = nc.scalar.dma_start(w2_sb[c][:], w2s[c])
                add_dep_helper(w2dma.ins, anchors[h].ins,
                               reason="delay w2 prefetch")
            elif h == 5 and TR > 0:
                wtdma = nc.scalar.dma_start(w13_sb[:], w13s[:])
                add_dep_helper(wtdma.ins, anchors[h].ins,
                               reason="delay w13 load")
                wtdmb = nc.scalar.dma_start(w13B_sb[:], w13sB[:])
                add_dep_helper(wtdmb.ins, anchors[h].ins,
                               reason="delay w13B load")
            elif h == H_PRE + 2 * KC - 1:
                gdma = nc.scalar.dma_start(g_sb[:], gs[:])
                add_dep_helper(gdma.ins, anchors[h].ins,
                               reason="delay g load")
                gcdma = nc.scalar.dma_start(gc_sb[:], gcs[:])
                add_dep_helper(gcdma.ins, anchors[h].ins,
                               reason="delay gcol load")

        # Fused tail pass: the last TR=40 h-rows of the w1 and w3 GEMMs
        # share ONE matmul chain ([40 p1-rows | 40 p3-rows] on partitions
        # 0..79, rest zero), saving a full 8-matmul chain per token tile vs
        # separate zero-padded 128-row tiles. silu(p1)*p3 needs partition-
        # ALIGNED operands, so the p3 half takes a tiny DRAM round trip to
        # realign partitions 40..79 -> 0..39 (DMA addressing shifts
        # partitions; engines cannot). All off the PE critical path --
        # phase B reads s_tl much later.
        if TR > 0:
            dramp = top.enter_context(
                tc.tile_pool(name="dram", bufs=1, space="DRAM"))
            sdr = dramp.tile([TW2, cap], BF16)
            ctp = top.enter_context(tc.tile_pool(name="ctp", bufs=2))
            dtp = top.enter_context(tc.tile_pool(name="dtp", bufs=1))
            # phase B's consumption order, so each s_tl lands well before
            # the B chain that contracts it; chains alternate between the
            # p1/p3 PSUM rings (p3 is idle here) so slot recycling never
            # waits on the trailing ACT silu+copy pipeline
            tail_order = sorted(range(len(tiles)), key=lambda i: -tiles[i][1])
            for ti, (to, tw) in enumerate([tiles[i] for i in tail_order]):
                wsel = w13B_sb if to == toB else w13_sb
                pt = psa.tile([128, TT], FP32,
                              tag="p1" if ti % 2 == 0 else "p3", name="pt")
                for k in range(KC):
                    nc.tensor.matmul(pt[:, :tw],
                                     wsel[:, k * 128:(k + 1) * 128],
                                     x_sb[:, k, to:to + tw],
                                     start=(k == 0), stop=(k == KC - 1))
                sa = sap.tile([128, TT], BF16, tag="sa", name="sat")
                nc.scalar.activation(sa[:TR, :tw], pt[:TR, :tw], AF.Silu)
                # the p3-half copy rides ACT too: the DVE stream's mults are
                # DMA-gated, and an in-order DVE copy behind them would hold
                # the PSUM slot recycle chain (and stall the PE tail chains)
                c3 = ctp.tile([TW2, TT], BF16, tag="c3", name="c3")
                nc.scalar.activation(c3[:, :tw], pt[:TW2, :tw], AF.Copy)
                nc.gpsimd.dma_start(sdr[:, to:to + tw], c3[:, :tw])
                d3 = dtp.tile([TR, TT], BF16, tag="d3", name="d3")
                nc.gpsimd.dma_start(d3[:, :tw], sdr[TR:TW2, to:to + tw])
                nc.vector.tensor_tensor(s_tl[:, to:to + tw], sa[:TR, :tw],
                                        d3[:, :tw], op=ALU.mult)

        # ---- phase B: y = (s @ w2.T) * g ----
        # Main tiles: full-width first, remainder LAST (short final drain).
        # segB's phase B runs TRANSPOSED (tokens on partitions, C on the
        # moving dim): lhsT = the s slices exactly as stored, rhs = w2B
        # streamed as cheap [128, C] h-tiles; its 44 ap-512 steps accumulate
        # into 2 persistent PSUM banks, interleaved one per main chain so
        # the stream keeps pace and everything hides under main-B compute.
        yp = top.enter_context(tc.tile_pool(name="yst", bufs=2))
        w2bp = top.enter_context(tc.tile_pool(name="w2bst", bufs=3))
        w2b_t = {}

        def load_w2b(h, dep=None):
            t_ = w2bp.tile([128, C], BF16, tag="w2b", name=f"w2b{h}")
            d = nc.gpsimd.dma_start(t_[:], w2bs[h])
            if dep is not None:
                add_dep_helper(d.ins, dep.ins, reason="delay w2b stream")
            w2b_t[h] = t_

        for h in range(min(3, KH)):
            load_w2b(h, dep=anchors[KHF - 1])

        def emit_segb_step(h):
            wt = w2b_t.pop(h)
            for half in range(2):
                cs = slice(half * TT, half * TT + TT)
                if h < KHF:
                    nc.tensor.matmul(pyB[half][:twB, :],
                                     s_sb[:, h, toB:toB + twB],
                                     wt[:, cs], start=(h == 0), stop=(h == KH - 1))
                else:
                    nc.tensor.matmul(pyB[half][:twB, :],
                                     s_tl[:, toB:toB + twB],
                                     wt[:TR, cs], start=False, stop=True)
            if h + 3 < KH:
                load_w2b(h + 3)

        b_order = sorted(range(nsa), key=lambda i: -tiles[i][1])
        bi = 0
        seq = [(t, c) for t in b_order for c in range(KC)]
        for i, (t, c) in enumerate(seq):
            to, tw = tiles[t]
            py = psb.tile([128, TT], FP32, tag="py", name=f"py{t}_{c}")
            for h in range(KHF):
                nc.tensor.matmul(py[:, :tw], w2_sb[c][:, h, :],
                                 s_sb[:, h, to:to + tw],
                                 start=(h == 0),
                                 stop=(TR == 0 and h == KHF - 1))
            if TR > 0:
                nc.tensor.matmul(py[:, :tw], w2_sb[c][:TR, KHF, :],
                                 s_tl[:, to:to + tw],
                                 start=False, stop=True)
            yb = yp.tile([128, TT], BF16, tag="y", name=f"yb{t}_{c}")
            nc.vector.tensor_tensor(yb[:, :tw], py[:, :tw],
                                    g_sb[:, to:to + tw], op=ALU.mult)
            nc.sync.dma_start(yt[c, :, to:to + tw], yb[:, :tw])
            if i >= KC and bi < KH:
                emit_segb_step(bi)
                bi += 1
                if bi == KH:
                    for half in range(2):
                        cs = slice(half * TT, half * TT + TT)
                        ybB = yp.tile([128, TT], BF16, tag="y",
                                      name=f"ybB{half}")
                        nc.vector.tensor_scalar(ybB[:twB, :],
                                                pyB[half][:twB, :],
                                                gc_sb[:twB, :], None, ALU.mult)
                        nc.sync.dma_start(ytB[:twB, cs], ybB[:twB, :])
        assert bi == KH, "segB steps did not fit into main B schedule"

    nc.compile()
    return nc


def kernel(x, gate_w, w1, w2, w3, top_k):
    global LAST_RESULTS
    x = np.asarray(x, dtype=np.float32)
    gw = np.asarray(gate_w, dtype=np.float32)
    w1 = np.asarray(w1, dtype=np.float32)
    w2 = np.asarray(w2, dtype=np.float32)
    w3 = np.asarray(w3, dtype=np.float32)
    assert int(np.asarray(top_k)) == 2
    Bb, T, Cc = x.shape
    N = Bb * T
    assert Cc == C and w1.shape == (E, H0, C)

    xf = np.ascontiguousarray(x.reshape(N, C))
    # Router on host (dispatch is the sharding step): top-2 selection via
    # stable argsort over fp32 logits (matches jax.lax.top_k here), softmax
    # combine weights in fp32.
    logits = xf @ gw.T
    order = np.argsort(-logits, axis=1, kind="stable")[:, :2]
    vals = np.take_along_axis(logits, order, axis=1)
    ex = np.exp(vals - vals.max(axis=1, keepdims=True))
    gweights = (ex / ex.sum(axis=1, keepdims=True)).astype(np.float32)
    tok, gval = [], []
    for e in range(E):
        sel = order == e                      # [N, 2]
        rows = sel.any(axis=1)
        idx = np.nonzero(rows)[0]
        slot = sel[idx, 1].astype(np.int64)   # 0 if top-1, 1 if top-2
        tok.append(idx)
        gval.append(gweights[idx, slot])

    wmaps = []
    for e in range(E):
        w1t = np.zeros((C, HP), np.float32)
        w1t[:, :H0] = w1[e].T
        w1s_np = np.ascontiguousarray(
            w1t.reshape(KC, 128, KH, 128).transpose(2, 1, 0, 3)
        ).reshape(KH, 128, C)[:KHF].astype(NP_BF16)
        w3t = np.zeros((C, HP), np.float32)
        w3t[:, :H0] = w3[e].T
        w3s_np = np.ascontiguousarray(
            w3t.reshape(KC, 128, KH, 128).transpose(2, 1, 0, 3)
        ).reshape(KH, 128, C)[:KHF].astype(NP_BF16)
        # fused tail: [TR w1-tail rows | TR w3-tail rows | zeros] as one
        # full-width lhsT panel
        cat = np.zeros((128, C), np.float32)
        cat[:TR] = w1[e][KHF * 128:H0]
        cat[TR:TW2] = w3[e][KHF * 128:H0]
        w13_np = np.ascontiguousarray(
            cat.T.reshape(KC, 128, 128).transpose(1, 0, 2)
        ).reshape(128, KC * 128).astype(NP_BF16)
        w2t = np.zeros((HP, C), np.float32)
        w2t[:H0] = w2[e].T
        w2s_np = np.ascontiguousarray(
            w2t.reshape(KH, 128, KC, 128).transpose(2, 1, 0, 3)
        ).reshape(KC, 128, KH * 128).astype(NP_BF16)
        w2bs_np = np.ascontiguousarray(
            w2t.reshape(KH, 128, C)).astype(NP_BF16)
        wmaps.append({"w1s": w1s_np, "w3s": w3s_np, "w13s": w13_np,
                      "w2s": w2s_np, "w2bs": w2bs_np})

    # --- segment plan: segA = s1 primary-expert tokens, segB = s2 tokens of
    # one (possibly foreign) expert. Chain splits at the boundary cost no PE
    # cycles, so capacity drops from max-load to ~mean-load. ---
    loads = [t.size for t in tok]
    s1 = min(loads)
    plan = None
    for s2 in range(8, 520, 8):
        if s1 + s2 > CAP_MAX:
            break
        ok = [min(n, s1) for n in loads]
        ob = [min(n - o, s2) for n, o in zip(loads, ok)]
        exc = [n - o - b for n, o, b in zip(loads, ok, ob)]
        free = [e for e in range(E) if loads[e] <= s1]
        chunks = []
        good = True
        for e in range(E):
            r = exc[e]
            while r > 0:
                chunks.append((e, min(r, s2)))
                r -= min(r, s2)
        if len(chunks) <= len(free):
            plan = (s2, ok, ob, chunks, free)
            break
    if plan is None:
        s1, s2 = max(loads), 8
        plan = (s2, [min(n, s1) for n in loads], [0] * E, [], [])
    s2, ok, ob, chunks, free = plan
    cap = s1 + s2

    # per-core segB contents: (expert, token indices)
    segB = []
    for e in range(E):
        segB.append([e, tok[e][ok[e]:ok[e] + ob[e]]])
    pos = [ok[e] + ob[e] for e in range(E)]
    for i, (q, ln) in enumerate(chunks):
        f = free[i]
        segB[f] = [q, tok[q][pos[q]:pos[q] + ln]]
        pos[q] += ln

    if (s1, s2) not in _CACHE:
        _CACHE[(s1, s2)] = _build(s1, s2)
    nc = _CACHE[(s1, s2)]

    out = np.zeros((N, C), np.float32)
    in_maps = []
    coreinfo = []
    for e in range(E):
        idxA = tok[e][:ok[e]]
        gA = gval[e][:ok[e]]
        eB, idxB = segB[e]
        gB = gval[eB][np.searchsorted(tok[eB], idxB)] if idxB.size else \
            np.zeros((0,), np.float32)
        xe = np.zeros((cap, C), np.float32)
        xe[:idxA.size] = xf[idxA]
        xe[s1:s1 + idxB.size] = xf[idxB]
        xs_np = np.ascontiguousarray(xe.T).reshape(KC, 128, cap).astype(NP_BF16)
        ge = np.zeros((cap,), np.float32)
        ge[:idxA.size] = gA
        gs_np = np.broadcast_to(ge.astype(NP_BF16), (128, cap)).copy()
        gc_np = np.zeros((128, 1), np.float32)
        gc_np[:idxB.size, 0] = gB
        wa, wb = wmaps[e], wmaps[eB]
        in_maps.append({
            "xs": xs_np, "gs": gs_np, "gcs": gc_np,
            "w1s": wa["w1s"], "w3s": wa["w3s"], "w13s": wa["w13s"],
            "w2s": wa["w2s"],
            "w1sB": wb["w1s"], "w3sB": wb["w3s"], "w13sB": wb["w13s"],
            "w2bs": wb["w2bs"]})
        coreinfo.append((idxA, idxB))

    trace = os.environ.get("BASS_MOE_TRACE", "0") == "1"
    try:
        res = run_bass_kernel_spmd(nc, in_maps, core_ids=list(range(E)),
                                   trace=trace)
    except ModuleNotFoundError:
        res = run_bass_kernel_spmd(nc, in_maps, core_ids=list(range(E)))
    LAST_RESULTS = res

    for e in range(E):
        idxA, idxB = coreinfo[e]
        ye = res.results[e]["yt"].astype(np.float32).reshape(C, cap).T
        out[idxA] += ye[:idxA.size]
        if idxB.size:
            yB = res.results[e]["ytB"].astype(np.float32)
            out[idxB] += yB[:idxB.size]
    return out.reshape(Bb, T, C)
